# Initial kernel scaffold
#
"""FEDformer encoder layer on 8 TRN2 NeuronCores — batch-data-parallel Bass kernel.

Strategy (self-contained; shapes hardcoded):
  B=16,L=2048,D=512,H=8,E=64,M=64,DFF=2048; 8 cores x 2 batches each; no collectives.

  Math restructuring (validated against the jax reference):
   - rfft+mode-gather == x @ Fcat where Fcat[l, 0:64]=cos(2*pi*k_j*l/L),
     Fcat[l, 64:128]=-sin(...), k_j = mode_index.
   - The q-projection (Wq) and out-projection (Wo) commute with the DFT, so they
     are applied in mode space ([128 x 512] instead of [2048 x 512] per batch;
     16x cheaper). k/v projections are dead code in the reference.
   - irfft of a spectrum with only bins 0..63 populated == P @ C2S2 where
     C2S2[0:64, t]=w_m cos(2*pi*m*t/L), C2S2[64:128, t]=-w_m sin(...),
     w_0=1/L, w_m=2/L  (Im of bin 0 drops automatically since sin(0)=0).
   - Fourier branch contributes ~1e-5 absolute to an O(1) output -> bf16 there.
   - series-decomp: K=2 softmax == sigmoid of weight/bias deltas; moving
     averages via one fp32 PADDED cumsum (13 head columns j*u0, scan seeded
     with 12*u0 via an AP initial, 12 tail extensions) so both windowed sums
     are single full-width subtracts with no edge fixups; ops software-
     pipelined across the 4 feature tiles to keep V and GP queues streaming.
   - FFN entirely in fp8e4 with DoubleRow matmuls (0.5 cyc/row): host
     prescales W1/W2 into e4m3 range; the scales fold into the gelu input
     scale and the psf drain. Residual r1 stays fp32 in mt; the FFN reads an
     fp8 copy in DoubleRow [128,2,L] layout (cast on the vector engine so the
     in-order scalar queue holds only sigmoids ahead of the FFN gelus). psf
     drains via scalar-engine copies + gpsimd adds so the FFN matmuls never
     queue behind decomposition vector work.

  Layout: device works feature-major ([D, L]); the host transposes x in and the
  output back during shard/unshard.
"""

import numpy as np

B, L, D, H, M, DFF = 16, 2048, 512, 8, 64, 2048
E = D // H
NC_ = 8
BLOC = B // NC_          # batches per core
MEXT = 2 * M             # re|im rows
NDC = D // 128           # 4 feature tiles
NFF = DFF // 128         # 16 dff tiles
NLC = L // 128           # 16 token chunks of 128
NTC = L // 512           # 4 token chunks of 512

_prog_cache = {}
_fixn = [0]


def _fix_sync_waits(nc, max_waits=1, max_updates=4):
    """Split >max sem-waits/updates per instruction onto adjacent nops.

    The AWS neuronx-cc walrus rejects instructions carrying too many sync
    commands ("Too many sync wait commands"); Tile's tail drain aggregates one
    wait per outstanding semaphore. Engine-order execution makes the split
    semantically identical.
    """
    import concourse.mybir as mybir

    for f in nc.m.functions:
        for bb in f.blocks:
            insts = bb.instructions
            i = 0
            while i < len(insts):
                ins = insts[i]
                si = ins.sync_info
                if si is not None and si.on_wait and len(si.on_wait) > max_waits:
                    waits = list(si.on_wait)
                    si.on_wait = waits[-max_waits:]
                    rest = waits[:-max_waits]
                    chunks = [rest[j:j + max_waits]
                              for j in range(0, len(rest), max_waits)]
                    for c in reversed(chunks):
                        _fixn[0] += 1
                        nop = mybir.InstNoOp(name=f"I-fixw-{_fixn[0]}", ins=[], outs=[])
                        nop.engine = ins.engine
                        nop.sync_info = mybir.SyncInfo(on_wait=c, on_update=[])
                        insts.insert(i, nop)
                        i += 1
                if si is not None and si.on_update and len(si.on_update) > max_updates:
                    ups = list(si.on_update)
                    si.on_update = ups[:max_updates]
                    rest = ups[max_updates:]
                    chunks = [rest[j:j + max_updates]
                              for j in range(0, len(rest), max_updates)]
                    for c in chunks:
                        _fixn[0] += 1
                        nop = mybir.InstNoOp(name=f"I-fixu-{_fixn[0]}", ins=[], outs=[])
                        nop.engine = ins.engine
                        nop.sync_info = mybir.SyncInfo(on_wait=[], on_update=c)
                        insts.insert(i + 1, nop)
                        i += 1
                i += 1


def _build_program(need_bq, j0, w2scale, w1scale, fix=True):
    import concourse.bass as bass
    import concourse.mybir as mybir
    from concourse.tile import TileContext

    F32 = mybir.dt.float32
    BF16 = mybir.dt.bfloat16
    FP8 = mybir.dt.float8e4
    W2SCALE = w2scale
    W1SCALE = w1scale
    AF = mybir.ActivationFunctionType
    OP = mybir.AluOpType

    nc = bass.Bass()

    # ---- DRAM I/O ----
    XT = nc.dram_tensor("XT", [BLOC, D, L], F32, kind="ExternalInput")
    XBF = nc.dram_tensor("XBF", [BLOC, 128, NLC * D], BF16, kind="ExternalInput")
    FCT = nc.dram_tensor("FCT", [128, NLC * 128], BF16, kind="ExternalInput")
    C2S2 = nc.dram_tensor("C2S2", [128, L], BF16, kind="ExternalInput")
    WQT = nc.dram_tensor("WQT", [D, D], BF16, kind="ExternalInput")
    WOT = nc.dram_tensor("WOT", [D, D], BF16, kind="ExternalInput")
    WPK = nc.dram_tensor("WPK", [H, 128, M * 128], BF16, kind="ExternalInput")
    W1T = nc.dram_tensor("W1T", [D, DFF], FP8, kind="ExternalInput")
    W2T = nc.dram_tensor("W2T", [DFF, D], FP8, kind="ExternalInput")
    EYE = nc.dram_tensor("EYE", [128, 128], BF16, kind="ExternalInput")
    BO4 = nc.dram_tensor("BO4", [128, NDC], F32, kind="ExternalInput")
    BQ4 = nc.dram_tensor("BQ4", [128, NDC], F32, kind="ExternalInput")
    ECHN = nc.dram_tensor("ECHN", [128, 13], F32, kind="ExternalInput")
    ETLN = nc.dram_tensor("ETLN", [128, 12], F32, kind="ExternalInput")
    DECS = nc.dram_tensor("DECS", [128, 4], F32, kind="ExternalInput")
    OUT_T = nc.dram_tensor("OUT_T", [BLOC, D, L], F32, kind="ExternalOutput")

    with TileContext(nc) as tc:
        # ---------- persistent pools (LIFO close at the end) ----------
        cst = tc.tile_pool(name="cst", bufs=1)
        cstp = cst.__enter__()
        main = tc.tile_pool(name="main", bufs=1)
        mainp = main.__enter__()
        decp_cm = tc.tile_pool(name="decp", bufs=1)
        decp = decp_cm.__enter__()
        r1cm = tc.tile_pool(name="r1p", bufs=1)
        r1p = r1cm.__enter__()

        # DMA issue order: what the front needs first.
        fct = cstp.tile([128, NLC * 128], BF16, name="fct")
        nc.sync.dma_start(out=fct[:], in_=FCT[:])
        wqt = [cstp.tile([128, D], BF16, name=f"wqt{i}") for i in range(NDC)]
        wot = [cstp.tile([128, D], BF16, name=f"wot{i}") for i in range(NDC)]
        eye = cstp.tile([128, 128], BF16, name="eye")
        c2s2 = cstp.tile([128, L], BF16, name="c2s2")
        bo4 = cstp.tile([128, NDC], F32, name="bo4")
        echn = cstp.tile([128, 13], F32, name="echn")
        etln = cstp.tile([128, 12], F32, name="etln")
        decs = cstp.tile([128, 4], F32, name="decs")
        bq4 = None

        mt = [[mainp.tile([128, L], F32, name=f"m_{b}_{dc}") for dc in range(NDC)]
              for b in range(BLOC)]
        r18 = [[r1p.tile([128, 2, L], FP8, name=f"r18_{b}_{kp}") for kp in range(2)]
               for b in range(BLOC)]

        # ---------- series decomposition (software-pipelined across dc) ----------
        def decomp_batch(b, dw_col, db_col, want_r1):
            """mt[b][*] (fp32 [128, L]) -> series-decomp residual, in place.

            Ops for the 4 feature tiles are issued in a skewed order so the
            vector queue never head-of-line blocks on gpsimd results: while
            tile k's combines wait on gpsimd, tile k+1's scan/band run.
            Only the cumsum is double-buffered; s13/s25/g reuse one buffer
            (engine-order makes their single-buffer reuse stall-free).
            """
            tl = {}

            def scan(dc):
                tl[dc] = dict(
                    u=mt[b][dc],
                    cs=decp.tile([128, L + 32], F32, name="cs", tag=f"cs{dc % 2}"),
                    s13=decp.tile([128, L], F32, name="s13", tag="s13"),
                    s25=decp.tile([128, L], F32, name="s25", tag="s25"),
                    g=decp.tile([128, L], BF16, name="g", tag=f"g{dc % 2}"),
                )
                t = tl[dc]
                u, cs, s25 = t["u"], t["cs"], t["s25"]
                # replicate-padded prefix sums: csp[j] = j*u0 (j<13), then the
                # inclusive scan seeded with 12*u0, then 12 tail extensions.
                nc.vector.tensor_scalar_mul(cs[:, 0:13], echn[:], u[:, 0:1])
                nc.vector.tensor_tensor_scan(cs[:, 13:2061], u[:], u[:],
                                             cs[:, 12:13], OP.add, OP.bypass)
                nc.vector.tensor_scalar(cs[:, 2061:2073], etln[:],
                                        u[:, 2047:2048], cs[:, 2060:2061],
                                        OP.mult, OP.add)
                nc.scalar.activation(t["g"][:], u[:], AF.Sigmoid,
                                     scale=decs[:, dw_col:dw_col + 1],
                                     bias=decs[:, db_col:db_col + 1])
                nc.gpsimd.tensor_tensor(s25[:], cs[:, 25:2073],
                                        cs[:, 0:2048], OP.subtract)

            def band(dc):
                t = tl[dc]
                nc.vector.tensor_tensor(t["s13"][:], t["cs"][:, 19:2067],
                                        t["cs"][:, 6:2054], OP.subtract)

            def combR(dc):
                # R = u - s25/25 (into cs; the cumsum is dead after the bands)
                t = tl[dc]
                nc.vector.scalar_tensor_tensor(
                    t["cs"][:, 0:2048], t["s25"][:], -1.0 / 25.0, t["u"][:],
                    OP.mult, OP.add)

            def delta(dc):
                # delta25 = s13*25/13 - s25 (into s25); q = g*delta25 (into g)
                t = tl[dc]
                nc.vector.scalar_tensor_tensor(
                    t["s25"][:], t["s13"][:], 25.0 / 13.0, t["s25"][:],
                    OP.mult, OP.subtract)
                nc.gpsimd.tensor_tensor(t["g"][:], t["g"][:], t["s25"][:], OP.mult)

            def combr(dc):
                # r = R - q/25 (into u)
                t = tl[dc]
                nc.vector.scalar_tensor_tensor(
                    t["u"][:], t["g"][:], -1.0 / 25.0, t["cs"][:, 0:2048],
                    OP.mult, OP.add)
                if want_r1:
                    # cast on V (not S): keeps the scalar queue free so the
                    # FFN gelus behind it in program order never wait on the
                    # next batch's decomposition.
                    nc.vector.tensor_copy(r18[b][dc // 2][:, dc % 2, :], t["u"][:])

            scan(0); band(0); combR(0); delta(0)
            scan(1); band(1); combr(0); combR(1); delta(1)
            scan(2); band(2); combr(1); combR(2); delta(2)
            scan(3); band(3); combr(2); combR(3); delta(3)
            combr(3)

        # ---------- Fourier branch (bf16) ----------
        with tc.tile_pool(name="fr", bufs=1) as fr, \
             tc.tile_pool(name="frp", bufs=1, space="PSUM") as frp, \
             tc.tile_pool(name="psmp", bufs=2, space="PSUM") as psmp, \
             tc.tile_pool(name="pswo", bufs=1, space="PSUM") as pswo, \
             tc.tile_pool(name="psy", bufs=2, space="PSUM") as psyp, \
             tc.tile_pool(name="wpkp", bufs=6) as wpkp, \
             tc.tile_pool(name="xbfp", bufs=4) as xbfp:
            # x token-major bf16, streamed in quarter-L chunks (4 buffers:
            # b1's first chunk can land as soon as b0's first drains)
            xbfq = {}
            for b in range(BLOC):
                for qc in range(4):
                    xbfq[(b, qc)] = xbfp.tile([128, 4 * D], BF16,
                                              name=f"xb{b}_{qc}", tag="xb")
            for b in range(BLOC):
                for qc in range(4):
                    nc.sync.dma_start(out=xbfq[(b, qc)][:],
                                      in_=XBF[b][:, qc * 4 * D:(qc + 1) * 4 * D])
            # remaining front weights
            nc.sync.dma_start(out=c2s2[:], in_=C2S2[:])
            for i in range(NDC):
                nc.sync.dma_start(out=wqt[i][:], in_=WQT[i * 128:(i + 1) * 128, :])
            nc.sync.dma_start(out=eye[:], in_=EYE[:])
            for t_, src in ((echn, ECHN), (etln, ETLN),
                            (decs, DECS), (bo4, BO4)):
                nc.sync.dma_start(out=t_[:], in_=src[:])
            if need_bq:
                bq4 = cstp.tile([128, NDC], F32, name="bq4")
                nc.sync.dma_start(out=bq4[:], in_=BQ4[:])
            for i in range(NDC):
                nc.sync.dma_start(out=wot[i][:], in_=WOT[i * 128:(i + 1) * 128, :])
            # x (feature-major fp32) for the residual stream
            for b in range(BLOC):
                for dc in range(NDC):
                    nc.sync.dma_start(out=mt[b][dc][:],
                                      in_=XT[b, dc * 128:(dc + 1) * 128, :])

            qt = [[None] * NDC for _ in range(BLOC)]
            for b in range(BLOC):
                # DFT: psD[m-ext, d] = sum_lc fct_lc^T @ xbf_lc   (16 matmuls, FD=512)
                psd = frp.tile([128, D], F32, name="psD", tag="psD")
                for lc in range(NLC):
                    nc.tensor.matmul(
                        psd[:], fct[:, lc * 128:(lc + 1) * 128],
                        xbfq[(b, lc // 4)][:, (lc % 4) * D:(lc % 4 + 1) * D],
                        start=(lc == 0), stop=(lc == NLC - 1))
                xsn = fr.tile([128, D], BF16, name=f"xsn{b}", tag="xsn")
                nc.scalar.copy(xsn[:], psd[:])
                # transpose to xselT[d, m-ext]
                xselT = fr.tile([128, NDC * 128], BF16, name=f"xselT{b}", tag="xselT")
                for dc in range(NDC):
                    pst = frp.tile([128, 128], BF16, name="psT", tag="psT")
                    nc.tensor.transpose(pst[:], xsn[:, dc * 128:(dc + 1) * 128],
                                        eye[:])
                    nc.scalar.copy(xselT[:, dc * 128:(dc + 1) * 128], pst[:])
                # q-projection in mode space: QT[dout, m-ext]
                for do in range(NDC):
                    qt[b][do] = fr.tile([128, 128], BF16, name=f"qt{b}_{do}",
                                        tag=f"qt{b}_{do}")
                    ps = frp.tile([128, 128], F32, name="psQ", tag="psQ")
                    for dc in range(NDC):
                        nc.tensor.matmul(
                            ps[:], wqt[dc][:, do * 128:(do + 1) * 128],
                            xselT[:, dc * 128:(dc + 1) * 128],
                            start=(dc == 0), stop=(dc == NDC - 1))
                    if need_bq:
                        nc.vector.tensor_tensor(
                            ps[:, j0:j0 + 1], ps[:, j0:j0 + 1],
                            bq4[:, do:do + 1], OP.add)
                    nc.scalar.copy(qt[b][do][:], ps[:])

            # mode mix: per head, per mode, complex ExE channel mix.
            # RH_h rows: 0:64 = Qre e-rows, 64:128 = Qim e-rows; col = 2m + b
            rh = [fr.tile([128, 128], BF16, name=f"rh{h}", tag=f"rh{h}")
                  for h in range(H)]
            for h in range(H):
                src_do, r0 = h // 2, (h % 2) * 64
                for b in range(BLOC):
                    rhv = rh[h].rearrange("p (m t) -> p m t", t=2)
                    nc.scalar.copy(rhv[0:64, :, b], qt[b][src_do][r0:r0 + 64, 0:64])
                    nc.scalar.copy(rhv[64:128, :, b], qt[b][src_do][r0:r0 + 64, 64:128])
            otre = [[fr.tile([128, M], BF16, name=f"otre{b}_{dc}", tag=f"otre{b}{dc}")
                     for dc in range(NDC)] for b in range(BLOC)]
            otim = [[fr.tile([128, M], BF16, name=f"otim{b}_{dc}", tag=f"otim{b}{dc}")
                     for dc in range(NDC)] for b in range(BLOC)]
            for h in range(H):
                psm = psmp.tile([128, 128], F32, name="psM", tag="psM")
                for q in range(4):
                    wpk_q = wpkp.tile([128, 16 * 128], BF16, name=f"wpk{h}_{q}",
                                      tag="wpk")
                    nc.sync.dma_start(out=wpk_q[:],
                                      in_=WPK[h][:, q * 2048:(q + 1) * 2048])
                    for mq in range(16):
                        m = q * 16 + mq
                        nc.tensor.matmul(
                            psm[:, 2 * m:2 * m + 2],
                            wpk_q[:, mq * 128:(mq + 1) * 128],
                            rh[h][:, 2 * m:2 * m + 2],
                            start=True, stop=True)
                psv = psm.rearrange("p (m t) -> p m t", t=2)
                dc, r0 = h // 2, (h % 2) * 64
                for b in range(BLOC):
                    nc.scalar.copy(otre[b][dc][r0:r0 + 64, :], psv[0:64, :, b])
                    nc.scalar.copy(otim[b][dc][r0:r0 + 64, :], psv[64:128, :, b])

            # x (feature-major fp32) for the residual stream (DMA behind wpk)
            for b in range(BLOC):
                for dc in range(NDC):
                    nc.sync.dma_start(out=mt[b][dc][:],
                                      in_=XT[b, dc * 128:(dc + 1) * 128, :])

            # Wo projection, transposed orientation: pcat[m-ext, dout] directly
            pcat = [fr.tile([128, D], BF16, name=f"pcat{b}", tag=f"pcat{b}")
                    for b in range(BLOC)]
            for b in range(BLOC):
                for ro, ot in ((0, otre[b]), (64, otim[b])):
                    psw = pswo.tile([M, D], F32, name="psW", tag="psW")
                    for dc in range(NDC):
                        nc.tensor.matmul(
                            psw[:], ot[dc][:], wot[dc][:],
                            start=(dc == 0), stop=(dc == NDC - 1))
                    nc.scalar.copy(pcat[b][ro:ro + 64, :], psw[:])

            # iDFT + u = x + yW + bo   (feature-major, fp32, in place over xT)
            for b in range(BLOC):
                for dc in range(NDC):
                    for t4 in range(NTC):
                        psy = psyp.tile([128, 512], F32, name="psY", tag="psY")
                        nc.tensor.matmul(
                            psy[:], pcat[b][:, dc * 128:(dc + 1) * 128],
                            c2s2[:, t4 * 512:(t4 + 1) * 512],
                            start=True, stop=True)
                        sl = mt[b][dc][:, t4 * 512:(t4 + 1) * 512]
                        nc.vector.scalar_tensor_tensor(
                            sl, psy[:], bo4[:, dc:dc + 1], sl, OP.add, OP.add)
            for b in range(BLOC):
                decomp_batch(b, 0, 1, True)

        # ---------- FFN (psh bf16; psf fp8e4 DoubleRow, host-prescaled W2) -----
        ffnw = tc.tile_pool(name="ffnw", bufs=1)
        ffnwp = ffnw.__enter__()
        w1dr = [ffnwp.tile([128, 2, DFF], FP8, name=f"w1dr{i}")
                for i in range(2)]
        for i in range(2):
            nc.sync.dma_start(out=w1dr[i][:, 0, :],
                              in_=W1T[(2 * i) * 128:(2 * i + 1) * 128, :])
            nc.sync.dma_start(out=w1dr[i][:, 1, :],
                              in_=W1T[(2 * i + 1) * 128:(2 * i + 2) * 128, :])
        w2dr = [ffnwp.tile([128, 2, D], FP8, name=f"w2dr{i}")
                for i in range(NFF // 2)]
        for i in range(NFF // 2):
            nc.sync.dma_start(out=w2dr[i][:, 0, :],
                              in_=W2T[(2 * i) * 128:(2 * i + 1) * 128, :])
            nc.sync.dma_start(out=w2dr[i][:, 1, :],
                              in_=W2T[(2 * i + 1) * 128:(2 * i + 2) * 128, :])

        with tc.tile_pool(name="gqp", bufs=3) as gqp, \
             tc.tile_pool(name="pshp", bufs=4, space="PSUM") as pshp, \
             tc.tile_pool(name="psfp", bufs=1, space="PSUM") as psfp:
            for b in range(BLOC):
                for t4 in range(NTC):
                    t0, t1 = t4 * 512, (t4 + 1) * 512
                    psf = [psfp.tile([128, 512], F32, name=f"psF{do}",
                                     tag=f"psF{do}") for do in range(NDC)]
                    for fp in range(NFF // 2):
                        gq2 = gqp.tile([128, 2, 512], FP8, name="gq2", tag="gq2")
                        for k in range(2):
                            ff = 2 * fp + k
                            psh = pshp.tile([128, 512], F32, name="psH", tag="psH")
                            for kp in range(2):
                                nc.tensor.matmul(
                                    psh[:],
                                    w1dr[kp][:, :, ff * 128:(ff + 1) * 128],
                                    r18[b][kp][:, :, t0:t1],
                                    start=(kp == 0), stop=(kp == 1),
                                    perf_mode=mybir.MatmulPerfMode.DoubleRow)
                            nc.scalar.activation(gq2[:, k, :], psh[:], AF.Gelu,
                                                 scale=1.0 / W1SCALE)
                        for do in range(NDC):
                            nc.tensor.matmul(
                                psf[do][:],
                                w2dr[fp][:, :, do * 128:(do + 1) * 128],
                                gq2[:, :, :],
                                start=(fp == 0), stop=(fp == NFF // 2 - 1),
                                perf_mode=mybir.MatmulPerfMode.DoubleRow)
                    # drain psf via scalar (scale folds the fp8 prescale) and
                    # add the residual on gpsimd -- keeps the FFN off the
                    # vector queue so it overlaps the decompositions.
                    # drain psf via scalar (scale folds the fp8 prescale);
                    # mt still holds r1 in fp32, add in place on gpsimd.
                    for do in range(NDC):
                        ftmp = gqp.tile([128, 512], F32, name="ftmp",
                                        tag=f"ftmp{do % 2}")
                        nc.scalar.activation(ftmp[:], psf[do][:], AF.Copy,
                                             scale=1.0 / W2SCALE)
                        nc.gpsimd.tensor_tensor(mt[b][do][:, t0:t1], ftmp[:],
                                                mt[b][do][:, t0:t1], OP.add)
                decomp_batch(b, 2, 3, False)
                for dc in range(NDC):
                    nc.sync.dma_start(out=OUT_T[b, dc * 128:(dc + 1) * 128, :],
                                      in_=mt[b][dc][:])

        ffnw.__exit__(None, None, None)
        r1cm.__exit__(None, None, None)
        decp_cm.__exit__(None, None, None)
        main.__exit__(None, None, None)
        cst.__exit__(None, None, None)

    if fix:
        _fix_sync_waits(nc)
    return nc


def _host_prep(inputs):
    import ml_dtypes
    bf16 = ml_dtypes.bfloat16
    x = np.asarray(inputs["x"], np.float32)
    w2t_pre = np.asarray(inputs["conv2_w"], np.float32).T
    w2scale = float(2.0 ** np.floor(np.log2(224.0 / np.abs(w2t_pre).max())))
    w1t_pre = np.asarray(inputs["conv1_w"], np.float32).T
    w1scale = float(2.0 ** np.floor(np.log2(224.0 / np.abs(w1t_pre).max())))
    modes = np.asarray(inputs["mode_index"]).astype(np.int64)
    l = np.arange(L, dtype=np.float64)
    ang = 2.0 * np.pi * np.outer(l, modes.astype(np.float64)) / L
    FC = np.concatenate([np.cos(ang), -np.sin(ang)], axis=1)          # [L, 128]
    m_out = np.arange(M, dtype=np.float64)
    w = np.where(m_out == 0, 1.0, 2.0) / L
    ang2 = 2.0 * np.pi * np.outer(m_out, l) / L
    C2 = np.concatenate([w[:, None] * np.cos(ang2),
                         w[:, None] * -np.sin(ang2)], axis=0)         # [128, L]

    FCT = FC.reshape(NLC, 128, 128).transpose(1, 0, 2).reshape(128, NLC * 128)

    wr = np.asarray(inputs["four_wr"], np.float64)   # [H, E, O, M]
    wi = np.asarray(inputs["four_wi"], np.float64)
    wpk = np.zeros((H, M, 128, 128), np.float64)
    wpk[:, :, 0:64, 0:64] = wr.transpose(0, 3, 1, 2)
    wpk[:, :, 0:64, 64:128] = wi.transpose(0, 3, 1, 2)
    wpk[:, :, 64:128, 0:64] = -wi.transpose(0, 3, 1, 2)
    wpk[:, :, 64:128, 64:128] = wr.transpose(0, 3, 1, 2)
    WPKh = wpk.transpose(0, 2, 1, 3).reshape(H, 128, M * 128)

    dec1_w = np.asarray(inputs["dec1_w"], np.float64)
    dec1_b = np.asarray(inputs["dec1_b"], np.float64)
    dec2_w = np.asarray(inputs["dec2_w"], np.float64)
    dec2_b = np.asarray(inputs["dec2_b"], np.float64)
    decs = np.zeros((128, 4), np.float32)
    decs[:, 0] = dec1_w[0] - dec1_w[1]
    decs[:, 1] = dec1_b[0] - dec1_b[1]
    decs[:, 2] = dec2_w[0] - dec2_w[1]
    decs[:, 3] = dec2_b[0] - dec2_b[1]

    bo = np.asarray(inputs["bo"], np.float32)
    bq = np.asarray(inputs["bq"], np.float32)
    BO4 = np.ascontiguousarray(bo.reshape(NDC, 128).T).astype(np.float32)
    zero_pos = np.nonzero(modes == 0)[0]
    need_bq = bool(len(zero_pos)) and bool(np.any(bq != 0))
    j0 = int(zero_pos[0]) if need_bq else 0
    BQ4 = np.ascontiguousarray((L * bq).reshape(NDC, 128).T).astype(np.float32)

    echn = np.tile(np.arange(13.0)[None, :], (128, 1)).astype(np.float32)
    etln = np.tile((np.arange(12.0) + 1.0)[None, :], (128, 1)).astype(np.float32)


    shared = {
        "FCT": FCT.astype(bf16),
        "C2S2": C2.astype(bf16),
        "WQT": np.ascontiguousarray(np.asarray(inputs["Wq"], np.float32).T).astype(bf16),
        "WOT": np.ascontiguousarray(np.asarray(inputs["Wo"], np.float32).T).astype(bf16),
        "WPK": WPKh.astype(bf16),
        "W1T": np.ascontiguousarray(w1t_pre * w1scale).astype(ml_dtypes.float8_e4m3),
        "W2T": np.ascontiguousarray(w2t_pre * w2scale).astype(ml_dtypes.float8_e4m3),
        "EYE": np.eye(128, dtype=np.float32).astype(bf16),
        "BO4": BO4, "BQ4": BQ4,
        "ECHN": echn, "ETLN": etln,
        "DECS": decs,
    }
    in_maps = []
    for c in range(NC_):
        xl = x[c * BLOC:(c + 1) * BLOC]                       # [2, L, D]
        XTc = np.ascontiguousarray(xl.transpose(0, 2, 1))     # [2, D, L]
        xbf = xl.astype(bf16)                                 # [2, L, D]
        XBFc = np.ascontiguousarray(
            xbf.reshape(BLOC, NLC, 128, D).transpose(0, 2, 1, 3)
        ).reshape(BLOC, 128, NLC * D)
        im = dict(shared)
        im["XT"] = XTc
        im["XBF"] = XBFc
        in_maps.append(im)
    return in_maps, need_bq, j0, w2scale, w1scale


def kernel(**inputs):
    from concourse.bass_utils import run_bass_kernel_spmd

    in_maps, need_bq, j0, w2scale, w1scale = _host_prep(inputs)
    key = (need_bq, j0, w2scale, w1scale)
    if key not in _prog_cache:
        _prog_cache[key] = _build_program(need_bq, j0, w2scale, w1scale)
    nc = _prog_cache[key]
    res = run_bass_kernel_spmd(nc, in_maps, core_ids=list(range(NC_)))
    outs = []
    for c in range(NC_):
        ot = np.asarray(res.results[c]["OUT_T"])              # [2, D, L]
        outs.append(np.ascontiguousarray(ot.transpose(0, 2, 1)))
    return np.concatenate(outs, axis=0).astype(np.float32)



# revision 1
# speedup vs baseline: 1.6235x; 1.6235x over previous
"""FEDformer encoder layer on 8 TRN2 NeuronCores — batch-data-parallel Bass kernel.

Strategy (self-contained; shapes hardcoded):
  B=16,L=2048,D=512,H=8,E=64,M=64,DFF=2048; 8 cores x 2 batches each; no collectives.

  Math restructuring (validated against the jax reference):
   - rfft+mode-gather == x @ Fcat where Fcat[l, 0:64]=cos(2*pi*k_j*l/L),
     Fcat[l, 64:128]=-sin(...), k_j = mode_index.
   - The q-projection (Wq) and out-projection (Wo) commute with the DFT, so they
     are applied in mode space ([128 x 512] instead of [2048 x 512] per batch;
     16x cheaper). k/v projections are dead code in the reference.
   - irfft of a spectrum with only bins 0..63 populated == P @ C2S2 where
     C2S2[0:64, t]=w_m cos(2*pi*m*t/L), C2S2[64:128, t]=-w_m sin(...),
     w_0=1/L, w_m=2/L  (Im of bin 0 drops automatically since sin(0)=0).
   - Fourier branch contributes ~1e-5 absolute to an O(1) output -> bf16 there.
   - series-decomp: K=2 softmax == sigmoid of weight/bias deltas; moving
     averages via one fp32 PADDED cumsum (13 head columns j*u0, scan seeded
     with 12*u0 via an AP initial, 12 tail extensions) so both windowed sums
     are single full-width subtracts with no edge fixups; ops software-
     pipelined across the 4 feature tiles to keep V and GP queues streaming.
   - FFN entirely in fp8e4 with DoubleRow matmuls (0.5 cyc/row): host
     prescales W1/W2 into e4m3 range; the scales fold into the gelu input
     scale and the psf drain. Residual r1 stays fp32 in mt; the FFN reads an
     fp8 copy in DoubleRow [128,2,L] layout (cast on the vector engine so the
     in-order scalar queue holds only sigmoids ahead of the FFN gelus). psf
     drains via scalar-engine copies + gpsimd adds so the FFN matmuls never
     queue behind decomposition vector work.

  Layout: device works feature-major ([D, L]); the host transposes x in and the
  output back during shard/unshard.
"""

import numpy as np

B, L, D, H, M, DFF = 16, 2048, 512, 8, 64, 2048
E = D // H
NC_ = 8
BLOC = B // NC_          # batches per core
MEXT = 2 * M             # re|im rows
NDC = D // 128           # 4 feature tiles
NFF = DFF // 128         # 16 dff tiles
NLC = L // 128           # 16 token chunks of 128
NTC = L // 512           # 4 token chunks of 512

_prog_cache = {}
_fixn = [0]


def _fix_sync_waits(nc, max_waits=1, max_updates=4):
    """Split >max sem-waits/updates per instruction onto adjacent nops.

    The AWS neuronx-cc walrus rejects instructions carrying too many sync
    commands ("Too many sync wait commands"); Tile's tail drain aggregates one
    wait per outstanding semaphore. Engine-order execution makes the split
    semantically identical.
    """
    import concourse.mybir as mybir

    for f in nc.m.functions:
        for bb in f.blocks:
            insts = bb.instructions
            i = 0
            while i < len(insts):
                ins = insts[i]
                si = ins.sync_info
                if si is not None and si.on_wait and len(si.on_wait) > max_waits:
                    waits = list(si.on_wait)
                    si.on_wait = waits[-max_waits:]
                    rest = waits[:-max_waits]
                    chunks = [rest[j:j + max_waits]
                              for j in range(0, len(rest), max_waits)]
                    for c in reversed(chunks):
                        _fixn[0] += 1
                        nop = mybir.InstNoOp(name=f"I-fixw-{_fixn[0]}", ins=[], outs=[])
                        nop.engine = ins.engine
                        nop.sync_info = mybir.SyncInfo(on_wait=c, on_update=[])
                        insts.insert(i, nop)
                        i += 1
                if si is not None and si.on_update and len(si.on_update) > max_updates:
                    ups = list(si.on_update)
                    si.on_update = ups[:max_updates]
                    rest = ups[max_updates:]
                    chunks = [rest[j:j + max_updates]
                              for j in range(0, len(rest), max_updates)]
                    for c in chunks:
                        _fixn[0] += 1
                        nop = mybir.InstNoOp(name=f"I-fixu-{_fixn[0]}", ins=[], outs=[])
                        nop.engine = ins.engine
                        nop.sync_info = mybir.SyncInfo(on_wait=[], on_update=c)
                        insts.insert(i + 1, nop)
                        i += 1
                i += 1


def _build_program(need_bq, j0, w2scale, w1scale, fix=True):
    import concourse.bass as bass
    import concourse.mybir as mybir
    from concourse.tile import TileContext

    F32 = mybir.dt.float32
    BF16 = mybir.dt.bfloat16
    FP8 = mybir.dt.float8e4
    W2SCALE = w2scale
    W1SCALE = w1scale
    AF = mybir.ActivationFunctionType
    OP = mybir.AluOpType

    nc = bass.Bass()

    # ---- DRAM I/O ----
    XT = nc.dram_tensor("XT", [BLOC, D, L], F32, kind="ExternalInput")
    XBF = nc.dram_tensor("XBF", [BLOC, 128, NLC * D], BF16, kind="ExternalInput")
    FCT = nc.dram_tensor("FCT", [128, NLC * 128], BF16, kind="ExternalInput")
    C2S2 = nc.dram_tensor("C2S2", [128, L], BF16, kind="ExternalInput")
    WQT = nc.dram_tensor("WQT", [D, D], BF16, kind="ExternalInput")
    WOT = nc.dram_tensor("WOT", [D, D], BF16, kind="ExternalInput")
    WPK = nc.dram_tensor("WPK", [H, 128, M * 128], BF16, kind="ExternalInput")
    W1T = nc.dram_tensor("W1T", [D, DFF], FP8, kind="ExternalInput")
    W2T = nc.dram_tensor("W2T", [DFF, D], FP8, kind="ExternalInput")
    EYE = nc.dram_tensor("EYE", [128, 128], BF16, kind="ExternalInput")
    BO4 = nc.dram_tensor("BO4", [128, NDC], F32, kind="ExternalInput")
    BQ4 = nc.dram_tensor("BQ4", [128, NDC], F32, kind="ExternalInput")
    ECHN = nc.dram_tensor("ECHN", [128, 13], F32, kind="ExternalInput")
    ETLN = nc.dram_tensor("ETLN", [128, 12], F32, kind="ExternalInput")
    DECS = nc.dram_tensor("DECS", [128, 4], F32, kind="ExternalInput")
    OUT_T = nc.dram_tensor("OUT_T", [BLOC, D, L], F32, kind="ExternalOutput")

    with TileContext(nc) as tc:
        # ---------- persistent pools (LIFO close at the end) ----------
        cst = tc.tile_pool(name="cst", bufs=1)
        cstp = cst.__enter__()
        main = tc.tile_pool(name="main", bufs=1)
        mainp = main.__enter__()
        decp_cm = tc.tile_pool(name="decp", bufs=1)
        decp = decp_cm.__enter__()
        r1cm = tc.tile_pool(name="r1p", bufs=1)
        r1p = r1cm.__enter__()

        # DMA issue order: what the front needs first.
        fct = cstp.tile([128, NLC * 128], BF16, name="fct")
        nc.sync.dma_start(out=fct[:], in_=FCT[:])
        wqt = [cstp.tile([128, D], BF16, name=f"wqt{i}") for i in range(NDC)]
        wot = [cstp.tile([128, D], BF16, name=f"wot{i}") for i in range(NDC)]
        eye = cstp.tile([128, 128], BF16, name="eye")
        c2s2 = cstp.tile([128, L], BF16, name="c2s2")
        bo4 = cstp.tile([128, NDC], F32, name="bo4")
        echn = cstp.tile([128, 13], F32, name="echn")
        etln = cstp.tile([128, 12], F32, name="etln")
        decs = cstp.tile([128, 4], F32, name="decs")
        bq4 = None

        mt = [[mainp.tile([128, L], F32, name=f"m_{b}_{dc}") for dc in range(NDC)]
              for b in range(BLOC)]
        r18 = [[r1p.tile([128, 2, L], FP8, name=f"r18_{b}_{kp}") for kp in range(2)]
               for b in range(BLOC)]

        # ---------- series decomposition (software-pipelined across dc) ----------
        def decomp_batch(b, dw_col, db_col, want_r1):
            """mt[b][*] (fp32 [128, L]) -> series-decomp residual, in place.

            Ops for the 4 feature tiles are issued in a skewed order so the
            vector queue never head-of-line blocks on gpsimd results: while
            tile k's combines wait on gpsimd, tile k+1's scan/band run.
            Only the cumsum is double-buffered; s13/s25/g reuse one buffer
            (engine-order makes their single-buffer reuse stall-free).
            """
            tl = {}

            def scan(dc):
                tl[dc] = dict(
                    u=mt[b][dc],
                    cs=decp.tile([128, L + 32], F32, name="cs", tag=f"cs{dc % 2}"),
                    s13=decp.tile([128, L], F32, name="s13", tag="s13"),
                    s25=decp.tile([128, L], F32, name="s25", tag="s25"),
                    g=decp.tile([128, L], BF16, name="g", tag=f"g{dc % 2}"),
                )
                t = tl[dc]
                u, cs, s25 = t["u"], t["cs"], t["s25"]
                # replicate-padded prefix sums: csp[j] = j*u0 (j<13), then the
                # inclusive scan seeded with 12*u0, then 12 tail extensions.
                nc.vector.tensor_scalar_mul(cs[:, 0:13], echn[:], u[:, 0:1])
                nc.vector.tensor_tensor_scan(cs[:, 13:2061], u[:], u[:],
                                             cs[:, 12:13], OP.add, OP.bypass)
                nc.vector.tensor_scalar(cs[:, 2061:2073], etln[:],
                                        u[:, 2047:2048], cs[:, 2060:2061],
                                        OP.mult, OP.add)
                nc.scalar.activation(t["g"][:], u[:], AF.Sigmoid,
                                     scale=decs[:, dw_col:dw_col + 1],
                                     bias=decs[:, db_col:db_col + 1])
                nc.gpsimd.tensor_tensor(s25[:], cs[:, 25:2073],
                                        cs[:, 0:2048], OP.subtract)

            def band(dc):
                t = tl[dc]
                nc.vector.tensor_tensor(t["s13"][:], t["cs"][:, 19:2067],
                                        t["cs"][:, 6:2054], OP.subtract)

            def combR(dc):
                # R = u - s25/25 (into cs; the cumsum is dead after the bands)
                t = tl[dc]
                nc.vector.scalar_tensor_tensor(
                    t["cs"][:, 0:2048], t["s25"][:], -1.0 / 25.0, t["u"][:],
                    OP.mult, OP.add)

            def delta(dc):
                # delta25 = s13*25/13 - s25 (into s25); q = g*delta25 (into g)
                t = tl[dc]
                nc.vector.scalar_tensor_tensor(
                    t["s25"][:], t["s13"][:], 25.0 / 13.0, t["s25"][:],
                    OP.mult, OP.subtract)
                nc.gpsimd.tensor_tensor(t["g"][:], t["g"][:], t["s25"][:], OP.mult)

            def combr(dc):
                # r = R - q/25 (into u)
                t = tl[dc]
                nc.vector.scalar_tensor_tensor(
                    t["u"][:], t["g"][:], -1.0 / 25.0, t["cs"][:, 0:2048],
                    OP.mult, OP.add)
                if want_r1:
                    # cast on V (not S): keeps the scalar queue free so the
                    # FFN gelus behind it in program order never wait on the
                    # next batch's decomposition.
                    nc.vector.tensor_copy(r18[b][dc // 2][:, dc % 2, :], t["u"][:])

            scan(0); band(0); combR(0); delta(0)
            scan(1); band(1); combr(0); combR(1); delta(1)
            scan(2); band(2); combr(1); combR(2); delta(2)
            scan(3); band(3); combr(2); combR(3); delta(3)
            combr(3)

        # ---------- Fourier branch (bf16) ----------
        with tc.tile_pool(name="fr", bufs=1) as fr, \
             tc.tile_pool(name="frp", bufs=1, space="PSUM") as frp, \
             tc.tile_pool(name="psmp", bufs=2, space="PSUM") as psmp, \
             tc.tile_pool(name="pswo", bufs=1, space="PSUM") as pswo, \
             tc.tile_pool(name="psy", bufs=2, space="PSUM") as psyp, \
             tc.tile_pool(name="wpkp", bufs=6) as wpkp, \
             tc.tile_pool(name="xbfp", bufs=4) as xbfp:
            # x token-major bf16, streamed in quarter-L chunks (4 buffers:
            # b1's first chunk can land as soon as b0's first drains)
            xbfq = {}
            for b in range(BLOC):
                for qc in range(4):
                    xbfq[(b, qc)] = xbfp.tile([128, 4 * D], BF16,
                                              name=f"xb{b}_{qc}", tag="xb")
            for b in range(BLOC):
                for qc in range(4):
                    nc.sync.dma_start(out=xbfq[(b, qc)][:],
                                      in_=XBF[b][:, qc * 4 * D:(qc + 1) * 4 * D])
            # remaining front weights
            nc.sync.dma_start(out=c2s2[:], in_=C2S2[:])
            for i in range(NDC):
                nc.sync.dma_start(out=wqt[i][:], in_=WQT[i * 128:(i + 1) * 128, :])
            nc.sync.dma_start(out=eye[:], in_=EYE[:])
            for t_, src in ((echn, ECHN), (etln, ETLN),
                            (decs, DECS), (bo4, BO4)):
                nc.sync.dma_start(out=t_[:], in_=src[:])
            if need_bq:
                bq4 = cstp.tile([128, NDC], F32, name="bq4")
                nc.sync.dma_start(out=bq4[:], in_=BQ4[:])
            for i in range(NDC):
                nc.sync.dma_start(out=wot[i][:], in_=WOT[i * 128:(i + 1) * 128, :])
            # x (feature-major fp32) for the residual stream
            for b in range(BLOC):
                for dc in range(NDC):
                    nc.sync.dma_start(out=mt[b][dc][:],
                                      in_=XT[b, dc * 128:(dc + 1) * 128, :])

            qt = [[None] * NDC for _ in range(BLOC)]
            for b in range(BLOC):
                # DFT: psD[m-ext, d] = sum_lc fct_lc^T @ xbf_lc   (16 matmuls, FD=512)
                psd = frp.tile([128, D], F32, name="psD", tag="psD")
                for lc in range(NLC):
                    nc.tensor.matmul(
                        psd[:], fct[:, lc * 128:(lc + 1) * 128],
                        xbfq[(b, lc // 4)][:, (lc % 4) * D:(lc % 4 + 1) * D],
                        start=(lc == 0), stop=(lc == NLC - 1))
                xsn = fr.tile([128, D], BF16, name=f"xsn{b}", tag="xsn")
                nc.scalar.copy(xsn[:], psd[:])
                # transpose to xselT[d, m-ext]
                xselT = fr.tile([128, NDC * 128], BF16, name=f"xselT{b}", tag="xselT")
                for dc in range(NDC):
                    pst = frp.tile([128, 128], BF16, name="psT", tag="psT")
                    nc.tensor.transpose(pst[:], xsn[:, dc * 128:(dc + 1) * 128],
                                        eye[:])
                    nc.scalar.copy(xselT[:, dc * 128:(dc + 1) * 128], pst[:])
                # q-projection in mode space: QT[dout, m-ext]
                for do in range(NDC):
                    qt[b][do] = fr.tile([128, 128], BF16, name=f"qt{b}_{do}",
                                        tag=f"qt{b}_{do}")
                    ps = frp.tile([128, 128], F32, name="psQ", tag="psQ")
                    for dc in range(NDC):
                        nc.tensor.matmul(
                            ps[:], wqt[dc][:, do * 128:(do + 1) * 128],
                            xselT[:, dc * 128:(dc + 1) * 128],
                            start=(dc == 0), stop=(dc == NDC - 1))
                    if need_bq:
                        nc.vector.tensor_tensor(
                            ps[:, j0:j0 + 1], ps[:, j0:j0 + 1],
                            bq4[:, do:do + 1], OP.add)
                    nc.scalar.copy(qt[b][do][:], ps[:])

            # mode mix: per head, per mode, complex ExE channel mix.
            # RH_h rows: 0:64 = Qre e-rows, 64:128 = Qim e-rows; col = 2m + b
            rh = [fr.tile([128, 128], BF16, name=f"rh{h}", tag=f"rh{h}")
                  for h in range(H)]
            for h in range(H):
                src_do, r0 = h // 2, (h % 2) * 64
                for b in range(BLOC):
                    rhv = rh[h].rearrange("p (m t) -> p m t", t=2)
                    nc.scalar.copy(rhv[0:64, :, b], qt[b][src_do][r0:r0 + 64, 0:64])
                    nc.scalar.copy(rhv[64:128, :, b], qt[b][src_do][r0:r0 + 64, 64:128])
            otre = [[fr.tile([128, M], BF16, name=f"otre{b}_{dc}", tag=f"otre{b}{dc}")
                     for dc in range(NDC)] for b in range(BLOC)]
            otim = [[fr.tile([128, M], BF16, name=f"otim{b}_{dc}", tag=f"otim{b}{dc}")
                     for dc in range(NDC)] for b in range(BLOC)]
            for h in range(H):
                psm = psmp.tile([128, 128], F32, name="psM", tag="psM")
                for q in range(4):
                    wpk_q = wpkp.tile([128, 16 * 128], BF16, name=f"wpk{h}_{q}",
                                      tag="wpk")
                    nc.sync.dma_start(out=wpk_q[:],
                                      in_=WPK[h][:, q * 2048:(q + 1) * 2048])
                    for mq in range(16):
                        m = q * 16 + mq
                        nc.tensor.matmul(
                            psm[:, 2 * m:2 * m + 2],
                            wpk_q[:, mq * 128:(mq + 1) * 128],
                            rh[h][:, 2 * m:2 * m + 2],
                            start=True, stop=True)
                psv = psm.rearrange("p (m t) -> p m t", t=2)
                dc, r0 = h // 2, (h % 2) * 64
                for b in range(BLOC):
                    nc.scalar.copy(otre[b][dc][r0:r0 + 64, :], psv[0:64, :, b])
                    nc.scalar.copy(otim[b][dc][r0:r0 + 64, :], psv[64:128, :, b])

            # x (feature-major fp32) for the residual stream (DMA behind wpk)
            for b in range(BLOC):
                for dc in range(NDC):
                    nc.sync.dma_start(out=mt[b][dc][:],
                                      in_=XT[b, dc * 128:(dc + 1) * 128, :])

            # Wo projection, transposed orientation: pcat[m-ext, dout] directly
            pcat = [fr.tile([128, D], BF16, name=f"pcat{b}", tag=f"pcat{b}")
                    for b in range(BLOC)]
            for b in range(BLOC):
                for ro, ot in ((0, otre[b]), (64, otim[b])):
                    psw = pswo.tile([M, D], F32, name="psW", tag="psW")
                    for dc in range(NDC):
                        nc.tensor.matmul(
                            psw[:], ot[dc][:], wot[dc][:],
                            start=(dc == 0), stop=(dc == NDC - 1))
                    nc.scalar.copy(pcat[b][ro:ro + 64, :], psw[:])

            # iDFT + u = x + yW + bo   (feature-major, fp32, in place over xT)
            for b in range(BLOC):
                for dc in range(NDC):
                    for t4 in range(NTC):
                        psy = psyp.tile([128, 512], F32, name="psY", tag="psY")
                        nc.tensor.matmul(
                            psy[:], pcat[b][:, dc * 128:(dc + 1) * 128],
                            c2s2[:, t4 * 512:(t4 + 1) * 512],
                            start=True, stop=True)
                        sl = mt[b][dc][:, t4 * 512:(t4 + 1) * 512]
                        nc.vector.scalar_tensor_tensor(
                            sl, psy[:], bo4[:, dc:dc + 1], sl, OP.add, OP.add)
            for b in range(BLOC):
                decomp_batch(b, 0, 1, True)

        # ---------- FFN (psh bf16; psf fp8e4 DoubleRow, host-prescaled W2) -----
        ffnw = tc.tile_pool(name="ffnw", bufs=1)
        ffnwp = ffnw.__enter__()
        w1dr = [ffnwp.tile([128, 2, DFF], FP8, name=f"w1dr{i}")
                for i in range(2)]
        for i in range(2):
            nc.sync.dma_start(out=w1dr[i][:, 0, :],
                              in_=W1T[(2 * i) * 128:(2 * i + 1) * 128, :])
            nc.sync.dma_start(out=w1dr[i][:, 1, :],
                              in_=W1T[(2 * i + 1) * 128:(2 * i + 2) * 128, :])
        w2dr = [ffnwp.tile([128, 2, D], FP8, name=f"w2dr{i}")
                for i in range(NFF // 2)]
        for i in range(NFF // 2):
            nc.sync.dma_start(out=w2dr[i][:, 0, :],
                              in_=W2T[(2 * i) * 128:(2 * i + 1) * 128, :])
            nc.sync.dma_start(out=w2dr[i][:, 1, :],
                              in_=W2T[(2 * i + 1) * 128:(2 * i + 2) * 128, :])

        with tc.tile_pool(name="gqp", bufs=3) as gqp, \
             tc.tile_pool(name="pshp", bufs=4, space="PSUM") as pshp, \
             tc.tile_pool(name="psfp", bufs=1, space="PSUM") as psfp:
            for b in range(BLOC):
                for t4 in range(NTC):
                    t0, t1 = t4 * 512, (t4 + 1) * 512
                    psf = [psfp.tile([128, 512], F32, name=f"psF{do}",
                                     tag=f"psF{do}") for do in range(NDC)]
                    for fp in range(NFF // 2):
                        gq2 = gqp.tile([128, 2, 512], FP8, name="gq2", tag="gq2")
                        for k in range(2):
                            ff = 2 * fp + k
                            psh = pshp.tile([128, 512], F32, name="psH", tag="psH")
                            for kp in range(2):
                                nc.tensor.matmul(
                                    psh[:],
                                    w1dr[kp][:, :, ff * 128:(ff + 1) * 128],
                                    r18[b][kp][:, :, t0:t1],
                                    start=(kp == 0), stop=(kp == 1),
                                    perf_mode=mybir.MatmulPerfMode.DoubleRow)
                            nc.scalar.activation(gq2[:, k, :], psh[:], AF.Gelu,
                                                 scale=1.0 / W1SCALE)
                        for do in range(NDC):
                            nc.tensor.matmul(
                                psf[do][:],
                                w2dr[fp][:, :, do * 128:(do + 1) * 128],
                                gq2[:, :, :],
                                start=(fp == 0), stop=(fp == NFF // 2 - 1),
                                perf_mode=mybir.MatmulPerfMode.DoubleRow)
                    # drain psf via scalar (scale folds the fp8 prescale) and
                    # add the residual on gpsimd -- keeps the FFN off the
                    # vector queue so it overlaps the decompositions.
                    # drain psf via scalar (scale folds the fp8 prescale);
                    # mt still holds r1 in fp32, add in place on gpsimd.
                    for do in range(NDC):
                        ftmp = gqp.tile([128, 512], F32, name="ftmp",
                                        tag=f"ftmp{do % 2}")
                        nc.scalar.activation(ftmp[:], psf[do][:], AF.Copy,
                                             scale=1.0 / W2SCALE)
                        nc.gpsimd.tensor_tensor(mt[b][do][:, t0:t1], ftmp[:],
                                                mt[b][do][:, t0:t1], OP.add)
                decomp_batch(b, 2, 3, False)
                for dc in range(NDC):
                    nc.sync.dma_start(out=OUT_T[b, dc * 128:(dc + 1) * 128, :],
                                      in_=mt[b][dc][:])

        ffnw.__exit__(None, None, None)
        r1cm.__exit__(None, None, None)
        decp_cm.__exit__(None, None, None)
        main.__exit__(None, None, None)
        cst.__exit__(None, None, None)

    if fix:
        _fix_sync_waits(nc)
    return nc


def _host_prep(inputs):
    import ml_dtypes
    bf16 = ml_dtypes.bfloat16
    x = np.asarray(inputs["x"], np.float32)
    w2t_pre = np.asarray(inputs["conv2_w"], np.float32).T
    w2scale = float(2.0 ** np.floor(np.log2(224.0 / np.abs(w2t_pre).max())))
    w1t_pre = np.asarray(inputs["conv1_w"], np.float32).T
    w1scale = float(2.0 ** np.floor(np.log2(224.0 / np.abs(w1t_pre).max())))
    modes = np.asarray(inputs["mode_index"]).astype(np.int64)
    l = np.arange(L, dtype=np.float64)
    ang = 2.0 * np.pi * np.outer(l, modes.astype(np.float64)) / L
    FC = np.concatenate([np.cos(ang), -np.sin(ang)], axis=1)          # [L, 128]
    m_out = np.arange(M, dtype=np.float64)
    w = np.where(m_out == 0, 1.0, 2.0) / L
    ang2 = 2.0 * np.pi * np.outer(m_out, l) / L
    C2 = np.concatenate([w[:, None] * np.cos(ang2),
                         w[:, None] * -np.sin(ang2)], axis=0)         # [128, L]

    FCT = FC.reshape(NLC, 128, 128).transpose(1, 0, 2).reshape(128, NLC * 128)

    wr = np.asarray(inputs["four_wr"], np.float64)   # [H, E, O, M]
    wi = np.asarray(inputs["four_wi"], np.float64)
    wpk = np.zeros((H, M, 128, 128), np.float64)
    wpk[:, :, 0:64, 0:64] = wr.transpose(0, 3, 1, 2)
    wpk[:, :, 0:64, 64:128] = wi.transpose(0, 3, 1, 2)
    wpk[:, :, 64:128, 0:64] = -wi.transpose(0, 3, 1, 2)
    wpk[:, :, 64:128, 64:128] = wr.transpose(0, 3, 1, 2)
    WPKh = wpk.transpose(0, 2, 1, 3).reshape(H, 128, M * 128)

    dec1_w = np.asarray(inputs["dec1_w"], np.float64)
    dec1_b = np.asarray(inputs["dec1_b"], np.float64)
    dec2_w = np.asarray(inputs["dec2_w"], np.float64)
    dec2_b = np.asarray(inputs["dec2_b"], np.float64)
    decs = np.zeros((128, 4), np.float32)
    decs[:, 0] = dec1_w[0] - dec1_w[1]
    decs[:, 1] = dec1_b[0] - dec1_b[1]
    decs[:, 2] = dec2_w[0] - dec2_w[1]
    decs[:, 3] = dec2_b[0] - dec2_b[1]

    bo = np.asarray(inputs["bo"], np.float32)
    bq = np.asarray(inputs["bq"], np.float32)
    BO4 = np.ascontiguousarray(bo.reshape(NDC, 128).T).astype(np.float32)
    zero_pos = np.nonzero(modes == 0)[0]
    need_bq = bool(len(zero_pos)) and bool(np.any(bq != 0))
    j0 = int(zero_pos[0]) if need_bq else 0
    BQ4 = np.ascontiguousarray((L * bq).reshape(NDC, 128).T).astype(np.float32)

    echn = np.tile(np.arange(13.0)[None, :], (128, 1)).astype(np.float32)
    etln = np.tile((np.arange(12.0) + 1.0)[None, :], (128, 1)).astype(np.float32)


    shared = {
        "FCT": FCT.astype(bf16),
        "C2S2": C2.astype(bf16),
        "WQT": np.ascontiguousarray(np.asarray(inputs["Wq"], np.float32).T).astype(bf16),
        "WOT": np.ascontiguousarray(np.asarray(inputs["Wo"], np.float32).T).astype(bf16),
        "WPK": WPKh.astype(bf16),
        "W1T": np.ascontiguousarray(w1t_pre * w1scale).astype(ml_dtypes.float8_e4m3),
        "W2T": np.ascontiguousarray(w2t_pre * w2scale).astype(ml_dtypes.float8_e4m3),
        "EYE": np.eye(128, dtype=np.float32).astype(bf16),
        "BO4": BO4, "BQ4": BQ4,
        "ECHN": echn, "ETLN": etln,
        "DECS": decs,
    }
    in_maps = []
    for c in range(NC_):
        xl = x[c * BLOC:(c + 1) * BLOC]                       # [2, L, D]
        XTc = np.ascontiguousarray(xl.transpose(0, 2, 1))     # [2, D, L]
        xbf = xl.astype(bf16)                                 # [2, L, D]
        XBFc = np.ascontiguousarray(
            xbf.reshape(BLOC, NLC, 128, D).transpose(0, 2, 1, 3)
        ).reshape(BLOC, 128, NLC * D)
        im = dict(shared)
        im["XT"] = XTc
        im["XBF"] = XBFc
        in_maps.append(im)
    return in_maps, need_bq, j0, w2scale, w1scale


def kernel(**inputs):
    from concourse.bass_utils import run_bass_kernel_spmd

    in_maps, need_bq, j0, w2scale, w1scale = _host_prep(inputs)
    key = (need_bq, j0, w2scale, w1scale)
    if key not in _prog_cache:
        _prog_cache[key] = _build_program(need_bq, j0, w2scale, w1scale)
    nc = _prog_cache[key]
    res = run_bass_kernel_spmd(nc, in_maps, core_ids=list(range(NC_)))
    outs = []
    for c in range(NC_):
        ot = np.asarray(res.results[c]["OUT_T"])              # [2, D, L]
        outs.append(np.ascontiguousarray(ot.transpose(0, 2, 1)))
    return np.concatenate(outs, axis=0).astype(np.float32)

